# revision 10
# baseline (speedup 1.0000x reference)
"""CrossViewAttention Trainium2 kernel.

Strategy: shard the Q=2500 query positions across 8 cores (Q padded to
2560 = 8*320). Softmax is over NK, which stays local per core, so no
collectives are needed. Per core everything runs in a "transposed"
layout: logits^T [NK_tile=128 partitions, Q=320 free] so that QK^T,
the softmax normalizer (one-hot ones-matmul), and attn@V all run on the
PE without any attention-matrix transposes. Masking uses
e = exp(logits*W*vis)*vis, which matches the reference's finfo.min
trick to float precision because logits are tiny. The softmax
normalizer is folded in after attn@V, and the per-head output
projection is accumulated directly from per-head tiles so no
partition-shifted engine ops are needed anywhere.
"""

import sys

if "/opt/trn_rl_repo" not in sys.path:
    sys.path.insert(0, "/opt/trn_rl_repo")

import numpy as np
import ml_dtypes

import concourse.bass as bass
import concourse.bacc as bacc_mod
import concourse.mybir as mybir
from concourse.tile import TileContext
from concourse.masks import make_identity

# problem constants (hardcoded per harness contract)
HEADS = 4
DH = 32
D = 128
EPS = 1e-5
HB = WB = 50
Q = HB * WB            # 2500
NVIEW, KH, KW = 6, 24, 44
NK = NVIEW * KH * KW   # 6336
NCORES = 8
QC = 320               # queries per core (Q padded to 2560)
QPAD = NCORES * QC
NKP = 6400             # NK padded to 50*128
NKT = NKP // 128       # 50 nk tiles
SCALE = DH ** -0.5

F32 = mybir.dt.float32
BF16 = mybir.dt.bfloat16
X = mybir.AxisListType.X
AF = mybir.ActivationFunctionType

_CACHE = {}


def _ln_partition_stats(nc, pool, pool1, ps_pool, ps_tag, pbc_pool, pbc_tag,
                        ones_col, ones_row, x_sbuf, out, g_ap, b_ap):
    """LayerNorm of x [128 partitions, Qf free] over the PARTITION dim.

    Column stats via ones-matmuls, broadcast back via K=1 matmuls, then
    out = ((x - mean) * rstd) * g + b with per-partition g/b on ACT.
    """
    Qf = x_sbuf.shape[-1]
    ps1 = ps_pool.tile([1, Qf], F32, tag=ps_tag)
    nc.tensor.matmul(ps1, ones_col, x_sbuf, start=True, stop=True)
    sq = pool1.tile([128, Qf], F32, tag="lnsq")
    ps2 = ps_pool.tile([1, Qf], F32, tag=ps_tag)
    nc.scalar.activation(sq, x_sbuf, AF.Square)
    nc.tensor.matmul(ps2, ones_col, sq, start=True, stop=True)
    mean = pool.tile([1, Qf], F32, tag="lnmean")
    ex2 = pool.tile([1, Qf], F32, tag="lnex2")
    nc.scalar.mul(mean, ps1, 1.0 / 128.0)
    nc.scalar.mul(ex2, ps2, 1.0 / 128.0)
    m2 = pool.tile([1, Qf], F32, tag="lnm2")
    nc.vector.tensor_mul(out=m2, in0=mean, in1=mean)
    var = pool.tile([1, Qf], F32, tag="lnvar")
    nc.vector.tensor_tensor(out=var, in0=ex2, in1=m2, op=mybir.AluOpType.subtract)
    std = pool.tile([1, Qf], F32, tag="lnstd")
    nc.scalar.activation(std, var, AF.Sqrt, bias=EPS)
    rstd = pool.tile([1, Qf], F32, tag="lnrstd")
    nc.vector.reciprocal(rstd, std)
    nmr = pool.tile([1, Qf], F32, tag="lnnmr")
    nc.vector.tensor_mul(out=nmr, in0=mean, in1=rstd)
    nc.scalar.mul(nmr, nmr, -1.0)
    pA = pbc_pool.tile([128, Qf], F32, tag=pbc_tag)
    pC = pbc_pool.tile([128, Qf], F32, tag=pbc_tag)
    nc.tensor.matmul(pA, ones_row, rstd, start=True, stop=True)
    nc.tensor.matmul(pC, ones_row, nmr, start=True, stop=True)
    t1 = pool1.tile([128, Qf], F32, tag="lnt1")
    nc.vector.tensor_mul(out=t1, in0=x_sbuf, in1=pA)
    t2 = pool1.tile([128, Qf], F32, tag="lnt2")
    nc.vector.tensor_add(out=t2, in0=t1, in1=pC)
    nc.scalar.activation(out, t2, AF.Identity, scale=g_ap, bias=b_ap)


def _build():
    if "nc" in _CACHE:
        return _CACHE["nc"]
    nc = bacc_mod.Bacc()

    # ---- I/O ----
    qT = nc.dram_tensor("qT", [D, QC], F32, kind="ExternalInput")
    kR = nc.dram_tensor("kR", [NKP, D], F32, kind="ExternalInput")
    vR = nc.dram_tensor("vR", [NKP, D], F32, kind="ExternalInput")
    Wt = nc.dram_tensor("Wt", [NKT, 128, QC], F32, kind="ExternalInput")
    Cm = nc.dram_tensor("Cm", [NKT, 128, QC], BF16, kind="ExternalInput")
    skipT = nc.dram_tensor("skipT", [D, QC], F32, kind="ExternalInput")
    wqT = nc.dram_tensor("wqT", [D, D], BF16, kind="ExternalInput")
    wkT = nc.dram_tensor("wkT", [D, D], BF16, kind="ExternalInput")
    wvT = nc.dram_tensor("wvT", [D, D], BF16, kind="ExternalInput")
    bqm = nc.dram_tensor("bqm", [64, 2], F32, kind="ExternalInput")
    bkm = nc.dram_tensor("bkm", [64, 2], F32, kind="ExternalInput")
    bvrow = nc.dram_tensor("bvrow", [1, D], BF16, kind="ExternalInput")
    wprojTm = nc.dram_tensor("wprojTm", [DH, HEADS, D], BF16, kind="ExternalInput")
    bprojv = nc.dram_tensor("bprojv", [D, 1], F32, kind="ExternalInput")
    pre_gv = nc.dram_tensor("pre_gv", [D, 1], F32, kind="ExternalInput")
    pre_bv = nc.dram_tensor("pre_bv", [D, 1], F32, kind="ExternalInput")
    w1T = nc.dram_tensor("w1T", [D, 2 * D], BF16, kind="ExternalInput")
    b1m = nc.dram_tensor("b1m", [D, 2], F32, kind="ExternalInput")
    w2Td = nc.dram_tensor("w2Td", [2, D, D], BF16, kind="ExternalInput")
    b2v = nc.dram_tensor("b2v", [D, 1], F32, kind="ExternalInput")
    post_gv = nc.dram_tensor("post_gv", [D, 1], F32, kind="ExternalInput")
    post_bv = nc.dram_tensor("post_bv", [D, 1], F32, kind="ExternalInput")
    outT = nc.dram_tensor("outT", [D, QC], F32, kind="ExternalOutput")

    with TileContext(nc) as tc:
        with tc.tile_pool(name="const", bufs=1) as cpool, \
             tc.tile_pool(name="big", bufs=1) as bigpool, \
             tc.tile_pool(name="work", bufs=3) as work, \
             tc.tile_pool(name="io", bufs=1) as io:

            # ---- constants ----
            ident = cpool.tile([128, 128], BF16)
            make_identity(nc, ident)
            ones_col = cpool.tile([128, 1], F32)
            nc.any.memset(ones_col, 1.0)
            ones_row = cpool.tile([1, 128], F32)
            nc.any.memset(ones_row, 1.0)
            ones_rbf = cpool.tile([1, 128], BF16)
            nc.any.memset(ones_rbf, 1.0)
            zero_c = cpool.tile([128, 1], F32)
            nc.any.memset(zero_c, 0.0)
            nc.const_aps.aps[(F32, 0.0)] = zero_c[:]
            eps_c = cpool.tile([128, 1], F32)
            nc.any.memset(eps_c, EPS)
            nc.const_aps.aps[(F32, EPS)] = eps_c[:]
            ones6432 = cpool.tile([64, 32], F32)
            nc.any.memset(ones6432, 1.0)

            def load_const(dram, shape, dt):
                t = cpool.tile(shape, dt, tag="c_" + dram.name)
                nc.sync.dma_start(t, dram[...])
                return t

            wq_s = load_const(wqT, [D, D], BF16)
            wk_s = load_const(wkT, [D, D], BF16)
            wv_s = load_const(wvT, [D, D], BF16)
            bq_s = load_const(bqm, [64, 2], F32)
            bk_s = load_const(bkm, [64, 2], F32)
            bvr_s = load_const(bvrow, [1, D], BF16)
            wproj_s = load_const(wprojTm, [DH, HEADS, D], BF16)
            bproj_s = load_const(bprojv, [D, 1], F32)
            preg_s = load_const(pre_gv, [D, 1], F32)
            preb_s = load_const(pre_bv, [D, 1], F32)
            w1_s = load_const(w1T, [D, 2 * D], BF16)
            b1_s = load_const(b1m, [D, 2], F32)
            w2_s = cpool.tile([D, 2, D], BF16)
            nc.sync.dma_start(w2_s[:, 0, :], w2Td[0])
            nc.sync.dma_start(w2_s[:, 1, :], w2Td[1])
            b2_s = load_const(b2v, [D, 1], F32)
            postg_s = load_const(post_gv, [D, 1], F32)
            postb_s = load_const(post_bv, [D, 1], F32)

            # ---- resident tensors ----
            # kfT/qfT split into lo/hi 64-partition halves so every
            # per-head [32, ...] slice has base partition 0 or 32.
            kf_lo = bigpool.tile([64, NKT, 128], BF16)
            kf_hi = bigpool.tile([64, NKT, 128], BF16)
            qf_lo = bigpool.tile([64, QC], BF16)
            qf_hi = bigpool.tile([64, QC], BF16)
            vf = bigpool.tile([128, NKT, HEADS, DH + 1], BF16)  # [nk,nkt,h,dh+1]
            nc.any.memset(vf[:, :, :, DH], 1.0)
            Wsb = bigpool.tile([128, NKT, QC], F32)
            Csb = bigpool.tile([128, NKT, QC], BF16)
            for t in range(NKT):
                nc.sync.dma_start(Wsb[:, t, :], Wt[t])
                nc.sync.dma_start(Csb[:, t, :], Cm[t])

            # ---- k/v prep (row LayerNorm + projection) ----
            with tc.tile_pool(name="psum_prep", bufs=2, space="PSUM") as ppre:
                for t in range(NKT):
                    for which in ("k", "v"):
                        src = kR if which == "k" else vR
                        raw = work.tile([128, D], F32, tag="kvraw")
                        nc.sync.dma_start(raw, src[t * 128:(t + 1) * 128, :])
                        s1 = work.tile([128, 1], F32, tag="s1")
                        nc.vector.reduce_sum(s1, raw, axis=X)
                        sq = work.tile([128, D], F32, tag="sq")
                        s2 = work.tile([128, 1], F32, tag="s2")
                        nc.scalar.activation(sq, raw, AF.Square, accum_out=s2)
                        mean = work.tile([128, 1], F32, tag="mean")
                        nc.scalar.mul(mean, s1, 1.0 / D)
                        ex2 = work.tile([128, 1], F32, tag="ex2")
                        nc.scalar.mul(ex2, s2, 1.0 / D)
                        m2 = work.tile([128, 1], F32, tag="m2")
                        nc.vector.tensor_mul(out=m2, in0=mean, in1=mean)
                        var = work.tile([128, 1], F32, tag="var")
                        nc.vector.tensor_tensor(out=var, in0=ex2, in1=m2,
                                                op=mybir.AluOpType.subtract)
                        std = work.tile([128, 1], F32, tag="std")
                        nc.scalar.activation(std, var, AF.Sqrt, bias=EPS)
                        rstd = work.tile([128, 1], F32, tag="rstd")
                        nc.vector.reciprocal(rstd, std)
                        nmr = work.tile([128, 1], F32, tag="nmr")
                        nc.vector.tensor_mul(out=nmr, in0=mean, in1=rstd)
                        nc.scalar.mul(nmr, nmr, -1.0)
                        norm = work.tile([128, D], BF16, tag="norm")
                        nc.scalar.activation(norm, raw, AF.Identity,
                                             scale=rstd, bias=nmr)
                        if which == "k":
                            pt = ppre.tile([128, 128], BF16, tag="pt")
                            nc.tensor.transpose(pt, norm, ident)
                            normT = work.tile([128, D], BF16, tag="normT")
                            nc.any.tensor_copy(out=normT, in_=pt)
                            pk_lo = ppre.tile([64, 128], F32, tag="pkv")
                            nc.tensor.matmul(pk_lo, wk_s[:, 0:64], normT,
                                             start=True, stop=True)
                            nc.scalar.activation(kf_lo[:, t, :], pk_lo,
                                                 AF.Identity,
                                                 bias=bk_s[:, 0:1])
                            pk_hi = ppre.tile([64, 128], F32, tag="pkv")
                            nc.tensor.matmul(pk_hi, wk_s[:, 64:128], normT,
                                             start=True, stop=True)
                            nc.scalar.activation(kf_hi[:, t, :], pk_hi,
                                                 AF.Identity,
                                                 bias=bk_s[:, 1:2])
                        else:
                            pt = ppre.tile([128, 128], BF16, tag="pt")
                            nc.tensor.transpose(pt, norm, ident)
                            normT = work.tile([128, D], BF16, tag="normT")
                            nc.any.tensor_copy(out=normT, in_=pt)
                            pv = ppre.tile([128, 128], F32, tag="pv")
                            nc.tensor.matmul(pv, normT, wv_s, start=True,
                                             stop=False)
                            nc.tensor.matmul(pv, ones_rbf, bvr_s, start=False,
                                             stop=True)
                            nc.any.tensor_copy(
                                out=vf[:, t, :, :DH],
                                in_=pv.rearrange("p (h e) -> p h e", h=HEADS))

            # ---- q prep ----
            with tc.tile_pool(name="psum_q", bufs=2, space="PSUM") as pqp:
                qsb = io.tile([D, QC], F32, tag="qsb")
                nc.sync.dma_start(qsb, qT[...])
                qn01 = work.tile([D, QC], BF16, tag="qn01")
                _ln_partition_stats(nc, work, io, pqp, "ps", pqp, "pbc",
                                    ones_col, ones_row, qsb, qn01, 1.0, 0.0)
                pq_lo = pqp.tile([64, QC], F32, tag="pq")
                nc.tensor.matmul(pq_lo, wq_s[:, 0:64], qn01, start=True, stop=True)
                nc.scalar.activation(qf_lo, pq_lo, AF.Identity, bias=bq_s[:, 0:1])
                pq_hi = pqp.tile([64, QC], F32, tag="pq")
                nc.tensor.matmul(pq_hi, wq_s[:, 64:128], qn01, start=True, stop=True)
                nc.scalar.activation(qf_hi, pq_hi, AF.Identity, bias=bq_s[:, 1:2])

            # ---- attention + projection + MLP ----
            with tc.tile_pool(name="psum_main", bufs=2, space="PSUM") as pmain, \
                 tc.tile_pool(name="psum_pl", bufs=3, space="PSUM") as pplp:
                pz = pplp.tile([128, QC], F32, tag="pl")
                for h in range(HEADS):
                    kf = (kf_lo, kf_hi)[h // 2]
                    qf = (qf_lo, qf_hi)[h // 2]
                    hb = 32 * (h % 2)
                    po = pmain.tile([DH + 1, QC], F32, tag="po")
                    for t in range(NKT):
                        pl = pplp.tile([128, QC], F32, tag="pl")
                        nc.tensor.matmul(pl, kf[hb:hb + 32, t, :],
                                         qf[hb:hb + 32, :],
                                         start=True, stop=True)
                        em = work.tile([128, QC], F32, tag="em")
                        nc.vector.tensor_mul(out=em, in0=pl, in1=Wsb[:, t, :])
                        ee = work.tile([128, QC], BF16, tag="ee")
                        nc.scalar.activation(ee, em, AF.Exp)
                        ec = work.tile([128, QC], BF16, tag="ec")
                        nc.vector.tensor_mul(out=ec, in0=ee, in1=Csb[:, t, :])
                        nc.tensor.matmul(po, vf[:, t, h, :], ec,
                                         start=(t == 0), stop=(t == NKT - 1))
                    # per-head normalize + projection accumulate
                    rt = work.tile([DH + 1, QC], F32, tag="rt")
                    nc.vector.reciprocal(rt[DH:DH + 1, :], po[DH:DH + 1, :])
                    prh = pmain.tile([DH, QC], F32, tag="prh")
                    nc.tensor.matmul(prh, ones6432[32:33, :], rt[DH:DH + 1, :],
                                     start=True, stop=True)
                    rbh = work.tile([DH, QC], F32, tag="rbh")
                    nc.any.tensor_copy(out=rbh, in_=prh)
                    onh = work.tile([DH, QC], BF16, tag="onh")
                    nc.vector.tensor_mul(out=onh, in0=po[:DH, :], in1=rbh)
                    nc.tensor.matmul(pz, wproj_s[:, h, :], onh,
                                     start=(h == 0), stop=(h == HEADS - 1))

                z0 = io.tile([D, QC], F32, tag="z0")
                nc.scalar.activation(z0, pz, AF.Identity, bias=bproj_s)
                sk = io.tile([D, QC], F32, tag="sk")
                nc.sync.dma_start(sk, skipT[...])
                z = io.tile([D, QC], F32, tag="z")
                nc.vector.tensor_add(out=z, in0=z0, in1=sk)

                zf = io.tile([D, QC], F32, tag="zf")
                _ln_partition_stats(nc, work, io, pmain, "prh", pmain, "po",
                                    ones_col, ones_row, z, zf, preg_s, preb_s)
                zfb = io.tile([D, QC], BF16, tag="zfb")
                nc.any.tensor_copy(out=zfb, in_=zf)

                h1 = io.tile([D, 2, QC], BF16, tag="h1")
                for j in range(2):
                    ph = pplp.tile([128, QC], F32, tag="pl")
                    nc.tensor.matmul(ph, w1_s[:, 128 * j:128 * (j + 1)], zfb,
                                     start=True, stop=True)
                    nc.scalar.activation(h1[:, j, :], ph, AF.Gelu,
                                         bias=b1_s[:, j:j + 1])
                pm = pplp.tile([128, QC], F32, tag="pl")
                nc.tensor.matmul(pm, w2_s[:, 0, :], h1[:, 0, :], start=True,
                                 stop=False)
                nc.tensor.matmul(pm, w2_s[:, 1, :], h1[:, 1, :], start=False,
                                 stop=True)
                z2 = io.tile([D, QC], F32, tag="z2")
                nc.scalar.activation(z2, pm, AF.Identity, bias=b2_s)
                z3 = io.tile([D, QC], F32, tag="z3")
                nc.vector.tensor_add(out=z3, in0=z2, in1=zf)

                zo = io.tile([D, QC], F32, tag="zo")
                _ln_partition_stats(nc, work, io, pmain, "prh", pmain, "po",
                                    ones_col, ones_row, z3, zo, postg_s, postb_s)
                nc.sync.dma_start(outT[...], zo)

    nc.finalize()
    _CACHE["nc"] = nc
    return nc


def _prep_inputs(inputs):
    f32 = np.float32
    bf16 = ml_dtypes.bfloat16
    q = np.asarray(inputs["q"], f32)
    k = np.asarray(inputs["k"], f32)
    v = np.asarray(inputs["v"], f32)
    W = np.asarray(inputs["W_logits"], f32)
    vis = np.asarray(inputs["vis"])
    skip = np.asarray(inputs["skip"], f32)

    g = lambda n: np.asarray(inputs[n], f32)
    qn_g, qn_b = g("qn_g"), g("qn_b")
    kn_g, kn_b = g("kn_g"), g("kn_b")
    vn_g, vn_b = g("vn_g"), g("vn_b")
    wq, bq = g("wq"), g("bq")
    wk, bk = g("wk"), g("bk")
    wv, bv = g("wv"), g("bv")
    wproj, bproj = g("wproj"), g("bproj")
    pre_g, pre_b = g("pre_g"), g("pre_b")
    w1, b1 = g("w1"), g("b1")
    w2, b2 = g("w2"), g("b2")
    post_g, post_b = g("post_g"), g("post_b")

    # fold LN affine params into projections; fold attention scale into q
    wq2 = (wq * qn_g[None, :]) * SCALE
    bq2 = (wq @ qn_b + bq) * SCALE
    wk2 = wk * kn_g[None, :]
    bk2 = wk @ kn_b + bk
    wv2 = wv * vn_g[None, :]
    bv2 = wv @ vn_b + bv

    # q/skip -> [D, Q] padded
    qT = np.zeros((D, QPAD), f32)
    qT[:, :Q] = q.reshape(D, Q)
    skipT = np.zeros((D, QPAD), f32)
    skipT[:, :Q] = skip.reshape(D, Q)

    # k/v -> rows [NKP, D]
    kRow = np.zeros((NKP, D), f32)
    kRow[:NK] = np.transpose(k, (0, 1, 3, 4, 2)).reshape(NK, D)
    vRow = np.zeros((NKP, D), f32)
    vRow[:NK] = np.transpose(v, (0, 1, 3, 4, 2)).reshape(NK, D)

    # W/vis -> transposed, padded; vis pad rows (queries) with 1 to avoid
    # a zero softmax denominator in the padding region
    Wp = np.zeros((QPAD, NKP), f32)
    Wp[:Q, :NK] = W[0]
    Cp = np.zeros((QPAD, NKP), f32)
    Cp[:Q, :NK] = vis[0]
    Cp[Q:, :] = 1.0

    # wproj head-major: wprojT [inner, D] -> [DH, HEADS, D]
    wprojT = np.ascontiguousarray(wproj.T)         # [inner, D]
    wprojTm = np.ascontiguousarray(
        wprojT.reshape(HEADS, DH, D).transpose(1, 0, 2))  # [DH, HEADS, D]

    shared = {
        "kR": kRow,
        "vR": vRow,
        "wqT": np.ascontiguousarray(wq2.T).astype(bf16),
        "wkT": np.ascontiguousarray(wk2.T).astype(bf16),
        "wvT": np.ascontiguousarray(wv2.T).astype(bf16),
        "bqm": np.ascontiguousarray(bq2.reshape(2, 64).T),
        "bkm": np.ascontiguousarray(bk2.reshape(2, 64).T),
        "bvrow": np.ascontiguousarray(bv2[None, :]).astype(bf16),
        "wprojTm": wprojTm.astype(bf16),
        "bprojv": np.ascontiguousarray(bproj[:, None]),
        "pre_gv": np.ascontiguousarray(pre_g[:, None]),
        "pre_bv": np.ascontiguousarray(pre_b[:, None]),
        "w1T": np.ascontiguousarray(w1.T).astype(bf16),
        "b1m": np.ascontiguousarray(b1.reshape(2, D).T),
        "w2Td": np.ascontiguousarray(w2.T.reshape(2, D, D)).astype(bf16),
        "b2v": np.ascontiguousarray(b2[:, None]),
        "post_gv": np.ascontiguousarray(post_g[:, None]),
        "post_bv": np.ascontiguousarray(post_b[:, None]),
    }

    in_maps = []
    for c in range(NCORES):
        sl = slice(c * QC, (c + 1) * QC)
        m = dict(shared)
        m["qT"] = np.ascontiguousarray(qT[:, sl])
        m["skipT"] = np.ascontiguousarray(skipT[:, sl])
        m["Wt"] = np.ascontiguousarray(Wp[sl].T).reshape(NKT, 128, QC)
        m["Cm"] = np.ascontiguousarray(Cp[sl].T).reshape(NKT, 128, QC).astype(bf16)
        in_maps.append(m)
    return in_maps


def kernel(**inputs):
    from concourse.bass_utils import run_bass_kernel_spmd

    nc = _build()
    in_maps = _prep_inputs(inputs)
    res = run_bass_kernel_spmd(nc, in_maps, core_ids=list(range(NCORES)))
    outs = np.concatenate([r["outT"] for r in res.results], axis=1)  # [D, QPAD]
    return outs[:, :Q].reshape(1, D, HB, WB).astype(np.float32)


# revision 13
# speedup vs baseline: 1.4153x; 1.4153x over previous
"""CrossViewAttention Trainium2 kernel.

Strategy: shard the Q=2500 query positions across 8 cores (Q padded to
2560 = 8*320). Softmax is over NK, which stays local per core, so no
collectives are needed. Per core everything runs in a "transposed"
layout: logits^T [NK_tile=128 partitions, Q=320 free] so that QK^T,
the softmax normalizer (one-hot ones-matmul), and attn@V all run on the
PE without any attention-matrix transposes. Masking uses
e = exp(logits*W*vis)*vis, which matches the reference's finfo.min
trick to float precision because logits are tiny. The softmax
normalizer is folded in after attn@V, and the per-head output
projection is accumulated directly from per-head tiles so no
partition-shifted engine ops are needed anywhere.
"""

import sys

if "/opt/trn_rl_repo" not in sys.path:
    sys.path.insert(0, "/opt/trn_rl_repo")

import numpy as np
import ml_dtypes

import concourse.bass as bass
import concourse.bacc as bacc_mod
import concourse.mybir as mybir
from concourse.tile import TileContext
from concourse.masks import make_identity

# problem constants (hardcoded per harness contract)
HEADS = 4
DH = 32
D = 128
EPS = 1e-5
HB = WB = 50
Q = HB * WB            # 2500
NVIEW, KH, KW = 6, 24, 44
NK = NVIEW * KH * KW   # 6336
NCORES = 8
QC = 320               # queries per core (Q padded to 2560)
QPAD = NCORES * QC
NKP = 6400             # NK padded to 50*128
NKT = NKP // 128       # 50 nk tiles
SCALE = DH ** -0.5

F32 = mybir.dt.float32
BF16 = mybir.dt.bfloat16
X = mybir.AxisListType.X
AF = mybir.ActivationFunctionType

_CACHE = {}


def _ln_partition_stats(nc, pool, pool1, ps_pool, ps_tag, pbc_pool, pbc_tag,
                        ones_col, ones_row, x_sbuf, out, g_ap, b_ap):
    """LayerNorm of x [128 partitions, Qf free] over the PARTITION dim.

    Column stats via ones-matmuls, broadcast back via K=1 matmuls, then
    out = ((x - mean) * rstd) * g + b with per-partition g/b on ACT.
    """
    Qf = x_sbuf.shape[-1]
    ps1 = ps_pool.tile([1, Qf], F32, tag=ps_tag)
    nc.tensor.matmul(ps1, ones_col, x_sbuf, start=True, stop=True)
    sq = pool1.tile([128, Qf], F32, tag="lnsq")
    ps2 = ps_pool.tile([1, Qf], F32, tag=ps_tag)
    nc.scalar.activation(sq, x_sbuf, AF.Square)
    nc.tensor.matmul(ps2, ones_col, sq, start=True, stop=True)
    mean = pool.tile([1, Qf], F32, tag="lnmean")
    ex2 = pool.tile([1, Qf], F32, tag="lnex2")
    nc.scalar.mul(mean, ps1, 1.0 / 128.0)
    nc.scalar.mul(ex2, ps2, 1.0 / 128.0)
    m2 = pool.tile([1, Qf], F32, tag="lnm2")
    nc.vector.tensor_mul(out=m2, in0=mean, in1=mean)
    var = pool.tile([1, Qf], F32, tag="lnvar")
    nc.vector.tensor_tensor(out=var, in0=ex2, in1=m2, op=mybir.AluOpType.subtract)
    std = pool.tile([1, Qf], F32, tag="lnstd")
    nc.scalar.activation(std, var, AF.Sqrt, bias=EPS)
    rstd = pool.tile([1, Qf], F32, tag="lnrstd")
    nc.vector.reciprocal(rstd, std)
    nmr = pool.tile([1, Qf], F32, tag="lnnmr")
    nc.vector.tensor_mul(out=nmr, in0=mean, in1=rstd)
    nc.scalar.mul(nmr, nmr, -1.0)
    pA = pbc_pool.tile([128, Qf], F32, tag=pbc_tag)
    pC = pbc_pool.tile([128, Qf], F32, tag=pbc_tag)
    nc.tensor.matmul(pA, ones_row, rstd, start=True, stop=True)
    nc.tensor.matmul(pC, ones_row, nmr, start=True, stop=True)
    t1 = pool1.tile([128, Qf], F32, tag="lnt1")
    nc.vector.tensor_mul(out=t1, in0=x_sbuf, in1=pA)
    t2 = pool1.tile([128, Qf], F32, tag="lnt2")
    nc.vector.tensor_add(out=t2, in0=t1, in1=pC)
    nc.scalar.activation(out, t2, AF.Identity, scale=g_ap, bias=b_ap)


def _build():
    if "nc" in _CACHE:
        return _CACHE["nc"]
    nc = bacc_mod.Bacc()

    # ---- I/O ----
    qT = nc.dram_tensor("qT", [D, QC], F32, kind="ExternalInput")
    kR = nc.dram_tensor("kR", [NKP, D], F32, kind="ExternalInput")
    vR = nc.dram_tensor("vR", [NKP, D], F32, kind="ExternalInput")
    Wt = nc.dram_tensor("Wt", [NKT, 128, QC], BF16, kind="ExternalInput")
    Cm = nc.dram_tensor("Cm", [NKT, 128, QC], BF16, kind="ExternalInput")
    skipT = nc.dram_tensor("skipT", [D, QC], F32, kind="ExternalInput")
    wqT = nc.dram_tensor("wqT", [D, D], BF16, kind="ExternalInput")
    wkT = nc.dram_tensor("wkT", [D, D], BF16, kind="ExternalInput")
    wvT = nc.dram_tensor("wvT", [D, D], BF16, kind="ExternalInput")
    bqm = nc.dram_tensor("bqm", [64, 2], F32, kind="ExternalInput")
    bkm = nc.dram_tensor("bkm", [64, 2], F32, kind="ExternalInput")
    bvrow = nc.dram_tensor("bvrow", [1, D], BF16, kind="ExternalInput")
    wprojTm = nc.dram_tensor("wprojTm", [DH, HEADS, D], BF16, kind="ExternalInput")
    bprojv = nc.dram_tensor("bprojv", [D, 1], F32, kind="ExternalInput")
    pre_gv = nc.dram_tensor("pre_gv", [D, 1], F32, kind="ExternalInput")
    pre_bv = nc.dram_tensor("pre_bv", [D, 1], F32, kind="ExternalInput")
    w1T = nc.dram_tensor("w1T", [D, 2 * D], BF16, kind="ExternalInput")
    b1m = nc.dram_tensor("b1m", [D, 2], F32, kind="ExternalInput")
    w2Td = nc.dram_tensor("w2Td", [2, D, D], BF16, kind="ExternalInput")
    b2v = nc.dram_tensor("b2v", [D, 1], F32, kind="ExternalInput")
    post_gv = nc.dram_tensor("post_gv", [D, 1], F32, kind="ExternalInput")
    post_bv = nc.dram_tensor("post_bv", [D, 1], F32, kind="ExternalInput")
    outT = nc.dram_tensor("outT", [D, QC], F32, kind="ExternalOutput")

    with TileContext(nc) as tc:
        with tc.tile_pool(name="const", bufs=1) as cpool, \
             tc.tile_pool(name="big", bufs=1) as bigpool, \
             tc.tile_pool(name="work", bufs=3) as work, \
             tc.tile_pool(name="io", bufs=1) as io:

            # ---- constants ----
            ident = cpool.tile([128, 128], BF16)
            make_identity(nc, ident)
            ones_col = cpool.tile([128, 1], F32)
            nc.any.memset(ones_col, 1.0)
            ones_row = cpool.tile([1, 128], F32)
            nc.any.memset(ones_row, 1.0)
            ones_rbf = cpool.tile([1, 128], BF16)
            nc.any.memset(ones_rbf, 1.0)
            zero_c = cpool.tile([128, 1], F32)
            nc.any.memset(zero_c, 0.0)
            nc.const_aps.aps[(F32, 0.0)] = zero_c[:]
            eps_c = cpool.tile([128, 1], F32)
            nc.any.memset(eps_c, EPS)
            nc.const_aps.aps[(F32, EPS)] = eps_c[:]
            ones6432 = cpool.tile([64, 32], F32)
            nc.any.memset(ones6432, 1.0)

            def load_const(dram, shape, dt):
                t = cpool.tile(shape, dt, tag="c_" + dram.name)
                nc.sync.dma_start(t, dram[...])
                return t

            wq_s = load_const(wqT, [D, D], BF16)
            wk_s = load_const(wkT, [D, D], BF16)
            wv_s = load_const(wvT, [D, D], BF16)
            bq_s = load_const(bqm, [64, 2], F32)
            bk_s = load_const(bkm, [64, 2], F32)
            bvr_s = load_const(bvrow, [1, D], BF16)
            wproj_s = load_const(wprojTm, [DH, HEADS, D], BF16)
            bproj_s = load_const(bprojv, [D, 1], F32)
            preg_s = load_const(pre_gv, [D, 1], F32)
            preb_s = load_const(pre_bv, [D, 1], F32)
            w1_s = load_const(w1T, [D, 2 * D], BF16)
            b1_s = load_const(b1m, [D, 2], F32)
            w2_s = cpool.tile([D, 2, D], BF16)
            nc.sync.dma_start(w2_s[:, 0, :], w2Td[0])
            nc.sync.dma_start(w2_s[:, 1, :], w2Td[1])
            b2_s = load_const(b2v, [D, 1], F32)
            postg_s = load_const(post_gv, [D, 1], F32)
            postb_s = load_const(post_bv, [D, 1], F32)

            # ---- resident tensors ----
            # kfT/qfT split into lo/hi 64-partition halves so every
            # per-head [32, ...] slice has base partition 0 or 32.
            kf_lo = bigpool.tile([64, NKT, 128], BF16)
            kf_hi = bigpool.tile([64, NKT, 128], BF16)
            qf_lo = bigpool.tile([64, QC], BF16)
            qf_hi = bigpool.tile([64, QC], BF16)
            vf = bigpool.tile([128, NKT, HEADS, DH + 1], BF16)  # [nk,nkt,h,dh+1]
            nc.any.memset(vf[:, :, :, DH], 1.0)
            Wsb = bigpool.tile([128, NKT, QC], BF16)
            Csb = bigpool.tile([128, NKT, QC], BF16)

            # ---- k/v prep (row LayerNorm + projection), chunked ----
            CH = 10
            with tc.tile_pool(name="psum_prep", bufs=2, space="PSUM") as ppre, \
                 tc.tile_pool(name="prep2", bufs=2) as prep2:
                for which in ("k", "v"):
                    src_d = kR if which == "k" else vR
                    for c0 in range(0, NKT, CH):
                        raw = prep2.tile([128, CH, D], F32, tag="kvraw")
                        nc.sync.dma_start(
                            raw, src_d[c0 * 128:(c0 + CH) * 128, :].rearrange(
                                "(t p) d -> p t d", p=128))
                        s1 = work.tile([128, CH], F32, tag="s1")
                        nc.vector.reduce_sum(s1, raw, axis=X)
                        sq = prep2.tile([128, CH, D], F32, tag="big_scratch")
                        nc.vector.tensor_mul(out=sq, in0=raw, in1=raw)
                        s2 = work.tile([128, CH], F32, tag="s2")
                        nc.vector.reduce_sum(s2, sq, axis=X)
                        meanN = work.tile([128, CH], F32, tag="meanN")
                        nc.scalar.mul(meanN, s1, -1.0 / D)
                        ex2 = work.tile([128, CH], F32, tag="ex2")
                        nc.scalar.mul(ex2, s2, 1.0 / D)
                        m2 = work.tile([128, CH], F32, tag="m2")
                        nc.vector.tensor_mul(out=m2, in0=meanN, in1=meanN)
                        var = work.tile([128, CH], F32, tag="var")
                        nc.vector.tensor_tensor(out=var, in0=ex2, in1=m2,
                                                op=mybir.AluOpType.subtract)
                        std = work.tile([128, CH], F32, tag="std")
                        nc.scalar.activation(std, var, AF.Sqrt, bias=EPS)
                        rstd = work.tile([128, CH], F32, tag="rstd")
                        nc.vector.reciprocal(rstd, std)
                        nmr = work.tile([128, CH], F32, tag="nmr")
                        nc.vector.tensor_mul(out=nmr, in0=meanN, in1=rstd)
                        t1 = prep2.tile([128, CH, D], F32, tag="big_scratch")
                        nc.vector.tensor_mul(
                            out=t1, in0=raw,
                            in1=rstd[:, :, None].to_broadcast((128, CH, D)))
                        kn = prep2.tile([128, CH, D], BF16, tag="knc")
                        nc.vector.tensor_add(
                            out=kn, in0=t1,
                            in1=nmr[:, :, None].to_broadcast((128, CH, D)))
                        for i in range(CH):
                            t = c0 + i
                            pt = ppre.tile([128, 128], BF16, tag="pt")
                            nc.tensor.transpose(pt, kn[:, i, :], ident)
                            normT = work.tile([128, D], BF16, tag="normT")
                            nc.any.tensor_copy(out=normT, in_=pt)
                            if which == "k":
                                pk_lo = ppre.tile([64, 128], F32, tag="pkv")
                                nc.tensor.matmul(pk_lo, wk_s[:, 0:64], normT,
                                                 start=True, stop=True)
                                nc.scalar.activation(kf_lo[:, t, :], pk_lo,
                                                     AF.Identity,
                                                     bias=bk_s[:, 0:1])
                                pk_hi = ppre.tile([64, 128], F32, tag="pkv")
                                nc.tensor.matmul(pk_hi, wk_s[:, 64:128], normT,
                                                 start=True, stop=True)
                                nc.scalar.activation(kf_hi[:, t, :], pk_hi,
                                                     AF.Identity,
                                                     bias=bk_s[:, 1:2])
                            else:
                                pv = ppre.tile([128, 128], F32, tag="pv")
                                nc.tensor.matmul(pv, normT, wv_s, start=True,
                                                 stop=False)
                                nc.tensor.matmul(pv, ones_rbf, bvr_s,
                                                 start=False, stop=True)
                                nc.any.tensor_copy(
                                    out=vf[:, t, :, :DH],
                                    in_=pv.rearrange("p (h e) -> p h e",
                                                     h=HEADS))

                # load the big mask tensors after prep DMAs are queued
                for t in range(NKT):
                    nc.sync.dma_start(Wsb[:, t, :], Wt[t])
                    nc.sync.dma_start(Csb[:, t, :], Cm[t])

            # ---- q prep ----
            with tc.tile_pool(name="psum_q", bufs=2, space="PSUM") as pqp:
                qsb = io.tile([D, QC], F32, tag="qsb")
                nc.sync.dma_start(qsb, qT[...])
                qn01 = work.tile([D, QC], BF16, tag="qn01")
                _ln_partition_stats(nc, work, io, pqp, "ps", pqp, "pbc",
                                    ones_col, ones_row, qsb, qn01, 1.0, 0.0)
                pq_lo = pqp.tile([64, QC], F32, tag="pq")
                nc.tensor.matmul(pq_lo, wq_s[:, 0:64], qn01, start=True, stop=True)
                nc.scalar.activation(qf_lo, pq_lo, AF.Identity, bias=bq_s[:, 0:1])
                pq_hi = pqp.tile([64, QC], F32, tag="pq")
                nc.tensor.matmul(pq_hi, wq_s[:, 64:128], qn01, start=True, stop=True)
                nc.scalar.activation(qf_hi, pq_hi, AF.Identity, bias=bq_s[:, 1:2])

            # ---- attention + projection + MLP ----
            with tc.tile_pool(name="psum_main", bufs=2, space="PSUM") as pmain, \
                 tc.tile_pool(name="psum_pl", bufs=3, space="PSUM") as pplp:
                pz = pplp.tile([128, QC], F32, tag="pl")
                for h in range(HEADS):
                    kf = (kf_lo, kf_hi)[h // 2]
                    qf = (qf_lo, qf_hi)[h // 2]
                    hb = 32 * (h % 2)
                    po = pmain.tile([DH + 1, QC], F32, tag="po")
                    for t in range(NKT):
                        pl = pplp.tile([128, QC], F32, tag="pl")
                        nc.tensor.matmul(pl, kf[hb:hb + 32, t, :],
                                         qf[hb:hb + 32, :],
                                         start=True, stop=True)
                        em = work.tile([128, QC], F32, tag="em")
                        nc.vector.tensor_mul(out=em, in0=pl, in1=Wsb[:, t, :])
                        ee = work.tile([128, QC], BF16, tag="ee")
                        nc.scalar.activation(ee, em, AF.Exp)
                        ec = work.tile([128, QC], BF16, tag="ec")
                        nc.gpsimd.tensor_mul(out=ec, in0=ee, in1=Csb[:, t, :])
                        nc.tensor.matmul(po, vf[:, t, h, :], ec,
                                         start=(t == 0), stop=(t == NKT - 1))
                    # per-head normalize + projection accumulate
                    rt = work.tile([DH + 1, QC], F32, tag="rt")
                    nc.vector.reciprocal(rt[DH:DH + 1, :], po[DH:DH + 1, :])
                    prh = pmain.tile([DH, QC], F32, tag="prh")
                    nc.tensor.matmul(prh, ones6432[32:33, :], rt[DH:DH + 1, :],
                                     start=True, stop=True)
                    rbh = work.tile([DH, QC], F32, tag="rbh")
                    nc.any.tensor_copy(out=rbh, in_=prh)
                    onh = work.tile([DH, QC], BF16, tag="onh")
                    nc.vector.tensor_mul(out=onh, in0=po[:DH, :], in1=rbh)
                    nc.tensor.matmul(pz, wproj_s[:, h, :], onh,
                                     start=(h == 0), stop=(h == HEADS - 1))

                z0 = io.tile([D, QC], F32, tag="z0")
                nc.scalar.activation(z0, pz, AF.Identity, bias=bproj_s)
                sk = io.tile([D, QC], F32, tag="sk")
                nc.sync.dma_start(sk, skipT[...])
                z = io.tile([D, QC], F32, tag="z")
                nc.vector.tensor_add(out=z, in0=z0, in1=sk)

                zf = io.tile([D, QC], F32, tag="zf")
                _ln_partition_stats(nc, work, io, pmain, "prh", pmain, "po",
                                    ones_col, ones_row, z, zf, preg_s, preb_s)
                zfb = io.tile([D, QC], BF16, tag="zfb")
                nc.any.tensor_copy(out=zfb, in_=zf)

                h1 = io.tile([D, 2, QC], BF16, tag="h1")
                for j in range(2):
                    ph = pplp.tile([128, QC], F32, tag="pl")
                    nc.tensor.matmul(ph, w1_s[:, 128 * j:128 * (j + 1)], zfb,
                                     start=True, stop=True)
                    nc.scalar.activation(h1[:, j, :], ph, AF.Gelu,
                                         bias=b1_s[:, j:j + 1])
                pm = pplp.tile([128, QC], F32, tag="pl")
                nc.tensor.matmul(pm, w2_s[:, 0, :], h1[:, 0, :], start=True,
                                 stop=False)
                nc.tensor.matmul(pm, w2_s[:, 1, :], h1[:, 1, :], start=False,
                                 stop=True)
                z2 = io.tile([D, QC], F32, tag="z2")
                nc.scalar.activation(z2, pm, AF.Identity, bias=b2_s)
                z3 = io.tile([D, QC], F32, tag="z3")
                nc.vector.tensor_add(out=z3, in0=z2, in1=zf)

                zo = io.tile([D, QC], F32, tag="zo")
                _ln_partition_stats(nc, work, io, pmain, "prh", pmain, "po",
                                    ones_col, ones_row, z3, zo, postg_s, postb_s)
                nc.sync.dma_start(outT[...], zo)

    nc.finalize()
    _CACHE["nc"] = nc
    return nc


def _prep_inputs(inputs):
    f32 = np.float32
    bf16 = ml_dtypes.bfloat16
    q = np.asarray(inputs["q"], f32)
    k = np.asarray(inputs["k"], f32)
    v = np.asarray(inputs["v"], f32)
    W = np.asarray(inputs["W_logits"], f32)
    vis = np.asarray(inputs["vis"])
    skip = np.asarray(inputs["skip"], f32)

    g = lambda n: np.asarray(inputs[n], f32)
    qn_g, qn_b = g("qn_g"), g("qn_b")
    kn_g, kn_b = g("kn_g"), g("kn_b")
    vn_g, vn_b = g("vn_g"), g("vn_b")
    wq, bq = g("wq"), g("bq")
    wk, bk = g("wk"), g("bk")
    wv, bv = g("wv"), g("bv")
    wproj, bproj = g("wproj"), g("bproj")
    pre_g, pre_b = g("pre_g"), g("pre_b")
    w1, b1 = g("w1"), g("b1")
    w2, b2 = g("w2"), g("b2")
    post_g, post_b = g("post_g"), g("post_b")

    # fold LN affine params into projections; fold attention scale into q
    wq2 = (wq * qn_g[None, :]) * SCALE
    bq2 = (wq @ qn_b + bq) * SCALE
    wk2 = wk * kn_g[None, :]
    bk2 = wk @ kn_b + bk
    wv2 = wv * vn_g[None, :]
    bv2 = wv @ vn_b + bv

    # q/skip -> [D, Q] padded
    qT = np.zeros((D, QPAD), f32)
    qT[:, :Q] = q.reshape(D, Q)
    skipT = np.zeros((D, QPAD), f32)
    skipT[:, :Q] = skip.reshape(D, Q)

    # k/v -> rows [NKP, D]
    kRow = np.zeros((NKP, D), f32)
    kRow[:NK] = np.transpose(k, (0, 1, 3, 4, 2)).reshape(NK, D)
    vRow = np.zeros((NKP, D), f32)
    vRow[:NK] = np.transpose(v, (0, 1, 3, 4, 2)).reshape(NK, D)

    # W/vis -> transposed, padded; vis pad rows (queries) with 1 to avoid
    # a zero softmax denominator in the padding region
    Wp = np.zeros((QPAD, NKP), f32)
    Wp[:Q, :NK] = W[0]
    Cp = np.zeros((QPAD, NKP), f32)
    Cp[:Q, :NK] = vis[0]
    Cp[Q:, :] = 1.0

    # wproj head-major: wprojT [inner, D] -> [DH, HEADS, D]
    wprojT = np.ascontiguousarray(wproj.T)         # [inner, D]
    wprojTm = np.ascontiguousarray(
        wprojT.reshape(HEADS, DH, D).transpose(1, 0, 2))  # [DH, HEADS, D]

    shared = {
        "kR": kRow,
        "vR": vRow,
        "wqT": np.ascontiguousarray(wq2.T).astype(bf16),
        "wkT": np.ascontiguousarray(wk2.T).astype(bf16),
        "wvT": np.ascontiguousarray(wv2.T).astype(bf16),
        "bqm": np.ascontiguousarray(bq2.reshape(2, 64).T),
        "bkm": np.ascontiguousarray(bk2.reshape(2, 64).T),
        "bvrow": np.ascontiguousarray(bv2[None, :]).astype(bf16),
        "wprojTm": wprojTm.astype(bf16),
        "bprojv": np.ascontiguousarray(bproj[:, None]),
        "pre_gv": np.ascontiguousarray(pre_g[:, None]),
        "pre_bv": np.ascontiguousarray(pre_b[:, None]),
        "w1T": np.ascontiguousarray(w1.T).astype(bf16),
        "b1m": np.ascontiguousarray(b1.reshape(2, D).T),
        "w2Td": np.ascontiguousarray(w2.T.reshape(2, D, D)).astype(bf16),
        "b2v": np.ascontiguousarray(b2[:, None]),
        "post_gv": np.ascontiguousarray(post_g[:, None]),
        "post_bv": np.ascontiguousarray(post_b[:, None]),
    }

    in_maps = []
    for c in range(NCORES):
        sl = slice(c * QC, (c + 1) * QC)
        m = dict(shared)
        m["qT"] = np.ascontiguousarray(qT[:, sl])
        m["skipT"] = np.ascontiguousarray(skipT[:, sl])
        m["Wt"] = np.ascontiguousarray(Wp[sl].T).reshape(NKT, 128, QC).astype(bf16)
        m["Cm"] = np.ascontiguousarray(Cp[sl].T).reshape(NKT, 128, QC).astype(bf16)
        in_maps.append(m)
    return in_maps


def kernel(**inputs):
    from concourse.bass_utils import run_bass_kernel_spmd

    nc = _build()
    in_maps = _prep_inputs(inputs)
    res = run_bass_kernel_spmd(nc, in_maps, core_ids=list(range(NCORES)))
    outs = np.concatenate([r["outT"] for r in res.results], axis=1)  # [D, QPAD]
    return outs[:, :Q].reshape(1, D, HB, WB).astype(np.float32)


# revision 14
# speedup vs baseline: 1.4735x; 1.0411x over previous
"""CrossViewAttention Trainium2 kernel.

Strategy: shard the Q=2500 query positions across 8 cores (Q padded to
2560 = 8*320). Softmax is over NK, which stays local per core, so no
collectives are needed. Per core everything runs in a "transposed"
layout: logits^T [NK_tile=128 partitions, Q=320 free] so that QK^T,
the softmax normalizer (one-hot ones-matmul), and attn@V all run on the
PE without any attention-matrix transposes. Masking uses
e = exp(logits*W*vis)*vis, which matches the reference's finfo.min
trick to float precision because logits are tiny. The softmax
normalizer is folded in after attn@V, and the per-head output
projection is accumulated directly from per-head tiles so no
partition-shifted engine ops are needed anywhere.
"""

import sys

if "/opt/trn_rl_repo" not in sys.path:
    sys.path.insert(0, "/opt/trn_rl_repo")

import numpy as np
import ml_dtypes

import concourse.bass as bass
import concourse.bacc as bacc_mod
import concourse.mybir as mybir
from concourse.tile import TileContext
from concourse.masks import make_identity

# problem constants (hardcoded per harness contract)
HEADS = 4
DH = 32
D = 128
EPS = 1e-5
HB = WB = 50
Q = HB * WB            # 2500
NVIEW, KH, KW = 6, 24, 44
NK = NVIEW * KH * KW   # 6336
NCORES = 8
QC = 320               # queries per core (Q padded to 2560)
QPAD = NCORES * QC
NKP = 6400             # NK padded to 50*128
NKT = NKP // 128       # 50 nk tiles
SCALE = DH ** -0.5

F32 = mybir.dt.float32
BF16 = mybir.dt.bfloat16
X = mybir.AxisListType.X
AF = mybir.ActivationFunctionType

_CACHE = {}


def _ln_partition_stats(nc, pool, pool1, ps_pool, ps_tag, pbc_pool, pbc_tag,
                        ones_col, ones_row, x_sbuf, out, g_ap, b_ap):
    """LayerNorm of x [128 partitions, Qf free] over the PARTITION dim.

    Column stats via ones-matmuls, broadcast back via K=1 matmuls, then
    out = ((x - mean) * rstd) * g + b with per-partition g/b on ACT.
    """
    Qf = x_sbuf.shape[-1]
    ps1 = ps_pool.tile([1, Qf], F32, tag=ps_tag)
    nc.tensor.matmul(ps1, ones_col, x_sbuf, start=True, stop=True)
    sq = pool1.tile([128, Qf], F32, tag="lnsq")
    ps2 = ps_pool.tile([1, Qf], F32, tag=ps_tag)
    nc.scalar.activation(sq, x_sbuf, AF.Square)
    nc.tensor.matmul(ps2, ones_col, sq, start=True, stop=True)
    mean = pool.tile([1, Qf], F32, tag="lnmean")
    ex2 = pool.tile([1, Qf], F32, tag="lnex2")
    nc.scalar.mul(mean, ps1, 1.0 / 128.0)
    nc.scalar.mul(ex2, ps2, 1.0 / 128.0)
    m2 = pool.tile([1, Qf], F32, tag="lnm2")
    nc.vector.tensor_mul(out=m2, in0=mean, in1=mean)
    var = pool.tile([1, Qf], F32, tag="lnvar")
    nc.vector.tensor_tensor(out=var, in0=ex2, in1=m2, op=mybir.AluOpType.subtract)
    std = pool.tile([1, Qf], F32, tag="lnstd")
    nc.scalar.activation(std, var, AF.Sqrt, bias=EPS)
    rstd = pool.tile([1, Qf], F32, tag="lnrstd")
    nc.vector.reciprocal(rstd, std)
    nmr = pool.tile([1, Qf], F32, tag="lnnmr")
    nc.vector.tensor_mul(out=nmr, in0=mean, in1=rstd)
    nc.scalar.mul(nmr, nmr, -1.0)
    pA = pbc_pool.tile([128, Qf], F32, tag=pbc_tag)
    pC = pbc_pool.tile([128, Qf], F32, tag=pbc_tag)
    nc.tensor.matmul(pA, ones_row, rstd, start=True, stop=True)
    nc.tensor.matmul(pC, ones_row, nmr, start=True, stop=True)
    t1 = pool1.tile([128, Qf], F32, tag="lnt1")
    nc.vector.tensor_mul(out=t1, in0=x_sbuf, in1=pA)
    t2 = pool1.tile([128, Qf], F32, tag="lnt2")
    nc.vector.tensor_add(out=t2, in0=t1, in1=pC)
    nc.scalar.activation(out, t2, AF.Identity, scale=g_ap, bias=b_ap)


def _build():
    if "nc" in _CACHE:
        return _CACHE["nc"]
    nc = bacc_mod.Bacc()

    # ---- I/O ----
    qT = nc.dram_tensor("qT", [D, QC], F32, kind="ExternalInput")
    kR = nc.dram_tensor("kR", [NKP, D], F32, kind="ExternalInput")
    vR = nc.dram_tensor("vR", [NKP, D], F32, kind="ExternalInput")
    Wt = nc.dram_tensor("Wt", [NKT, 128, QC], BF16, kind="ExternalInput")
    Cm = nc.dram_tensor("Cm", [NKT, 128, QC], BF16, kind="ExternalInput")
    skipT = nc.dram_tensor("skipT", [D, QC], F32, kind="ExternalInput")
    wqT = nc.dram_tensor("wqT", [D, D], BF16, kind="ExternalInput")
    wkT = nc.dram_tensor("wkT", [D, D], BF16, kind="ExternalInput")
    wvT = nc.dram_tensor("wvT", [D, D], BF16, kind="ExternalInput")
    bqm = nc.dram_tensor("bqm", [64, 2], F32, kind="ExternalInput")
    bkm = nc.dram_tensor("bkm", [64, 2], F32, kind="ExternalInput")
    wprojTm = nc.dram_tensor("wprojTm", [DH, HEADS, D], BF16, kind="ExternalInput")
    bprojv = nc.dram_tensor("bprojv", [D, 1], F32, kind="ExternalInput")
    pre_gv = nc.dram_tensor("pre_gv", [D, 1], F32, kind="ExternalInput")
    pre_bv = nc.dram_tensor("pre_bv", [D, 1], F32, kind="ExternalInput")
    w1T = nc.dram_tensor("w1T", [D, 2 * D], BF16, kind="ExternalInput")
    b1m = nc.dram_tensor("b1m", [D, 2], F32, kind="ExternalInput")
    w2Td = nc.dram_tensor("w2Td", [2, D, D], BF16, kind="ExternalInput")
    b2v = nc.dram_tensor("b2v", [D, 1], F32, kind="ExternalInput")
    post_gv = nc.dram_tensor("post_gv", [D, 1], F32, kind="ExternalInput")
    post_bv = nc.dram_tensor("post_bv", [D, 1], F32, kind="ExternalInput")
    outT = nc.dram_tensor("outT", [D, QC], F32, kind="ExternalOutput")

    with TileContext(nc) as tc:
        with tc.tile_pool(name="const", bufs=1) as cpool, \
             tc.tile_pool(name="big", bufs=1) as bigpool, \
             tc.tile_pool(name="work", bufs=3) as work, \
             tc.tile_pool(name="io", bufs=1) as io:

            # ---- constants ----
            ident = cpool.tile([128, 128], BF16)
            make_identity(nc, ident)
            ones_col = cpool.tile([128, 1], F32)
            nc.any.memset(ones_col, 1.0)
            ones_row = cpool.tile([1, 128], F32)
            nc.any.memset(ones_row, 1.0)
            ones_rbf = cpool.tile([1, 128], BF16)
            nc.any.memset(ones_rbf, 1.0)
            zero_c = cpool.tile([128, 1], F32)
            nc.any.memset(zero_c, 0.0)
            nc.const_aps.aps[(F32, 0.0)] = zero_c[:]
            eps_c = cpool.tile([128, 1], F32)
            nc.any.memset(eps_c, EPS)
            nc.const_aps.aps[(F32, EPS)] = eps_c[:]
            ones6432 = cpool.tile([64, 32], F32)
            nc.any.memset(ones6432, 1.0)

            def load_const(dram, shape, dt):
                t = cpool.tile(shape, dt, tag="c_" + dram.name)
                nc.sync.dma_start(t, dram[...])
                return t

            wq_s = load_const(wqT, [D, D], BF16)
            wk_s = load_const(wkT, [D, D], BF16)
            wv_s = load_const(wvT, [D, D], BF16)
            bq_s = load_const(bqm, [64, 2], F32)
            bk_s = load_const(bkm, [64, 2], F32)
            wproj_s = load_const(wprojTm, [DH, HEADS, D], BF16)
            bproj_s = load_const(bprojv, [D, 1], F32)
            preg_s = load_const(pre_gv, [D, 1], F32)
            preb_s = load_const(pre_bv, [D, 1], F32)
            w1_s = load_const(w1T, [D, 2 * D], BF16)
            b1_s = load_const(b1m, [D, 2], F32)
            w2_s = cpool.tile([D, 2, D], BF16)
            nc.sync.dma_start(w2_s[:, 0, :], w2Td[0])
            nc.sync.dma_start(w2_s[:, 1, :], w2Td[1])
            b2_s = load_const(b2v, [D, 1], F32)
            postg_s = load_const(post_gv, [D, 1], F32)
            postb_s = load_const(post_bv, [D, 1], F32)

            # ---- resident tensors ----
            # kfT/qfT split into lo/hi 64-partition halves so every
            # per-head [32, ...] slice has base partition 0 or 32.
            kf_lo = bigpool.tile([64, NKT, 128], BF16)
            kf_hi = bigpool.tile([64, NKT, 128], BF16)
            qf_lo = bigpool.tile([64, QC], BF16)
            qf_hi = bigpool.tile([64, QC], BF16)
            vf = bigpool.tile([128, NKT, HEADS, DH + 1], BF16)  # [nk,nkt,h,dh+1]
            nc.any.memset(vf[:, :, :, DH], 1.0)
            Wsb = bigpool.tile([128, NKT, QC], BF16)
            Csb = bigpool.tile([128, NKT, QC], BF16)

            # ---- k/v prep (row LayerNorm + projection), chunked ----
            CH = 10
            with tc.tile_pool(name="psum_prep", bufs=2, space="PSUM") as ppre, \
                 tc.tile_pool(name="prep2", bufs=2) as prep2:
                for which in ("k", "v"):
                    src_d = kR if which == "k" else vR
                    for c0 in range(0, NKT, CH):
                        raw = prep2.tile([128, CH, D], F32, tag="kvraw")
                        nc.sync.dma_start(
                            raw, src_d[c0 * 128:(c0 + CH) * 128, :].rearrange(
                                "(t p) d -> p t d", p=128))
                        s1 = work.tile([128, CH], F32, tag="s1")
                        nc.vector.reduce_sum(s1, raw, axis=X)
                        sq = prep2.tile([128, CH, D], F32, tag="big_scratch")
                        nc.vector.tensor_mul(out=sq, in0=raw, in1=raw)
                        s2 = work.tile([128, CH], F32, tag="s2")
                        nc.vector.reduce_sum(s2, sq, axis=X)
                        meanN = work.tile([128, CH], F32, tag="meanN")
                        nc.scalar.mul(meanN, s1, -1.0 / D)
                        ex2 = work.tile([128, CH], F32, tag="ex2")
                        nc.scalar.mul(ex2, s2, 1.0 / D)
                        m2 = work.tile([128, CH], F32, tag="m2")
                        nc.vector.tensor_mul(out=m2, in0=meanN, in1=meanN)
                        var = work.tile([128, CH], F32, tag="var")
                        nc.vector.tensor_tensor(out=var, in0=ex2, in1=m2,
                                                op=mybir.AluOpType.subtract)
                        std = work.tile([128, CH], F32, tag="std")
                        nc.scalar.activation(std, var, AF.Sqrt, bias=EPS)
                        rstd = work.tile([128, CH], F32, tag="rstd")
                        nc.vector.reciprocal(rstd, std)
                        nmr = work.tile([128, CH], F32, tag="nmr")
                        nc.vector.tensor_mul(out=nmr, in0=meanN, in1=rstd)
                        t1 = prep2.tile([128, CH, D], F32, tag="big_scratch")
                        nc.vector.tensor_mul(
                            out=t1, in0=raw,
                            in1=rstd[:, :, None].to_broadcast((128, CH, D)))
                        kn = prep2.tile([128, CH, D], BF16, tag="knc")
                        nc.vector.tensor_add(
                            out=kn, in0=t1,
                            in1=nmr[:, :, None].to_broadcast((128, CH, D)))
                        for i in range(CH):
                            t = c0 + i
                            pt = ppre.tile([128, 128], BF16, tag="pt")
                            nc.tensor.transpose(pt, kn[:, i, :], ident)
                            normT = work.tile([128, D], BF16, tag="normT")
                            nc.any.tensor_copy(out=normT, in_=pt)
                            if which == "k":
                                pk_lo = ppre.tile([64, 128], F32, tag="pkv")
                                nc.tensor.matmul(pk_lo, wk_s[:, 0:64], normT,
                                                 start=True, stop=True)
                                nc.scalar.activation(kf_lo[:, t, :], pk_lo,
                                                     AF.Identity,
                                                     bias=bk_s[:, 0:1])
                                pk_hi = ppre.tile([64, 128], F32, tag="pkv")
                                nc.tensor.matmul(pk_hi, wk_s[:, 64:128], normT,
                                                 start=True, stop=True)
                                nc.scalar.activation(kf_hi[:, t, :], pk_hi,
                                                     AF.Identity,
                                                     bias=bk_s[:, 1:2])
                            else:
                                pv = ppre.tile([128, 128], F32, tag="pv")
                                nc.tensor.matmul(pv, normT, wv_s, start=True,
                                                 stop=True)
                                nc.any.tensor_copy(
                                    out=vf[:, t, :, :DH],
                                    in_=pv.rearrange("p (h e) -> p h e",
                                                     h=HEADS))

                # load the big mask tensors after prep DMAs are queued
                for t in range(NKT):
                    nc.sync.dma_start(Wsb[:, t, :], Wt[t])
                    nc.sync.dma_start(Csb[:, t, :], Cm[t])

            # ---- q prep ----
            with tc.tile_pool(name="psum_q", bufs=2, space="PSUM") as pqp:
                qsb = io.tile([D, QC], F32, tag="qsb")
                nc.sync.dma_start(qsb, qT[...])
                qn01 = work.tile([D, QC], BF16, tag="qn01")
                _ln_partition_stats(nc, work, io, pqp, "ps", pqp, "pbc",
                                    ones_col, ones_row, qsb, qn01, 1.0, 0.0)
                pq_lo = pqp.tile([64, QC], F32, tag="pq")
                nc.tensor.matmul(pq_lo, wq_s[:, 0:64], qn01, start=True, stop=True)
                nc.scalar.activation(qf_lo, pq_lo, AF.Identity, bias=bq_s[:, 0:1])
                pq_hi = pqp.tile([64, QC], F32, tag="pq")
                nc.tensor.matmul(pq_hi, wq_s[:, 64:128], qn01, start=True, stop=True)
                nc.scalar.activation(qf_hi, pq_hi, AF.Identity, bias=bq_s[:, 1:2])

            # ---- attention + projection + MLP ----
            with tc.tile_pool(name="psum_main", bufs=2, space="PSUM") as pmain, \
                 tc.tile_pool(name="psum_pl", bufs=4, space="PSUM") as pplp:
                pz = pmain.tile([128, QC], F32, tag="prh")
                for h in range(HEADS):
                    kf = (kf_lo, kf_hi)[h // 2]
                    qf = (qf_lo, qf_hi)[h // 2]
                    hb = 32 * (h % 2)
                    po = pmain.tile([DH + 1, QC], F32, tag="po")
                    for t in range(NKT):
                        pl = pplp.tile([128, QC], F32, tag="pl")
                        nc.tensor.matmul(pl, kf[hb:hb + 32, t, :],
                                         qf[hb:hb + 32, :],
                                         start=True, stop=True)
                        em = work.tile([128, QC], F32, tag="em")
                        nc.vector.tensor_mul(out=em, in0=pl, in1=Wsb[:, t, :])
                        ee = work.tile([128, QC], BF16, tag="ee")
                        nc.scalar.activation(ee, em, AF.Exp)
                        ec = work.tile([128, QC], BF16, tag="ec")
                        nc.gpsimd.tensor_mul(out=ec, in0=ee, in1=Csb[:, t, :])
                        nc.tensor.matmul(po, vf[:, t, h, :], ec,
                                         start=(t == 0), stop=(t == NKT - 1))
                    # per-head normalize + projection accumulate
                    rt = work.tile([DH + 1, QC], F32, tag="rt")
                    nc.vector.reciprocal(rt[DH:DH + 1, :], po[DH:DH + 1, :])
                    prh = pmain.tile([DH, QC], F32, tag="prh")
                    nc.tensor.matmul(prh, ones6432[32:33, :], rt[DH:DH + 1, :],
                                     start=True, stop=True)
                    rbh = work.tile([DH, QC], F32, tag="rbh")
                    nc.any.tensor_copy(out=rbh, in_=prh)
                    onh = work.tile([DH, QC], BF16, tag="onh")
                    nc.vector.tensor_mul(out=onh, in0=po[:DH, :], in1=rbh)
                    nc.tensor.matmul(pz, wproj_s[:, h, :], onh,
                                     start=(h == 0), stop=(h == HEADS - 1))

                z0 = io.tile([D, QC], F32, tag="z0")
                nc.scalar.activation(z0, pz, AF.Identity, bias=bproj_s)
                sk = io.tile([D, QC], F32, tag="sk")
                nc.sync.dma_start(sk, skipT[...])
                z = io.tile([D, QC], F32, tag="z")
                nc.vector.tensor_add(out=z, in0=z0, in1=sk)

                zf = io.tile([D, QC], F32, tag="zf")
                _ln_partition_stats(nc, work, io, pmain, "prh", pmain, "po",
                                    ones_col, ones_row, z, zf, preg_s, preb_s)
                zfb = io.tile([D, QC], BF16, tag="zfb")
                nc.any.tensor_copy(out=zfb, in_=zf)

                h1 = io.tile([D, 2, QC], BF16, tag="h1")
                for j in range(2):
                    ph = pplp.tile([128, QC], F32, tag="pl")
                    nc.tensor.matmul(ph, w1_s[:, 128 * j:128 * (j + 1)], zfb,
                                     start=True, stop=True)
                    nc.scalar.activation(h1[:, j, :], ph, AF.Gelu,
                                         bias=b1_s[:, j:j + 1])
                pm = pplp.tile([128, QC], F32, tag="pl")
                nc.tensor.matmul(pm, w2_s[:, 0, :], h1[:, 0, :], start=True,
                                 stop=False)
                nc.tensor.matmul(pm, w2_s[:, 1, :], h1[:, 1, :], start=False,
                                 stop=True)
                z2 = io.tile([D, QC], F32, tag="z2")
                nc.scalar.activation(z2, pm, AF.Identity, bias=b2_s)
                z3 = io.tile([D, QC], F32, tag="z3")
                nc.vector.tensor_add(out=z3, in0=z2, in1=zf)

                zo = io.tile([D, QC], F32, tag="zo")
                _ln_partition_stats(nc, work, io, pmain, "prh", pmain, "po",
                                    ones_col, ones_row, z3, zo, postg_s, postb_s)
                nc.sync.dma_start(outT[...], zo)

    nc.finalize()
    _CACHE["nc"] = nc
    return nc


def _prep_inputs(inputs):
    f32 = np.float32
    bf16 = ml_dtypes.bfloat16
    q = np.asarray(inputs["q"], f32)
    k = np.asarray(inputs["k"], f32)
    v = np.asarray(inputs["v"], f32)
    W = np.asarray(inputs["W_logits"], f32)
    vis = np.asarray(inputs["vis"])
    skip = np.asarray(inputs["skip"], f32)

    g = lambda n: np.asarray(inputs[n], f32)
    qn_g, qn_b = g("qn_g"), g("qn_b")
    kn_g, kn_b = g("kn_g"), g("kn_b")
    vn_g, vn_b = g("vn_g"), g("vn_b")
    wq, bq = g("wq"), g("bq")
    wk, bk = g("wk"), g("bk")
    wv, bv = g("wv"), g("bv")
    wproj, bproj = g("wproj"), g("bproj")
    pre_g, pre_b = g("pre_g"), g("pre_b")
    w1, b1 = g("w1"), g("b1")
    w2, b2 = g("w2"), g("b2")
    post_g, post_b = g("post_g"), g("post_b")

    # fold LN affine params into projections; fold attention scale into q
    wq2 = (wq * qn_g[None, :]) * SCALE
    bq2 = (wq @ qn_b + bq) * SCALE
    wk2 = wk * kn_g[None, :]
    bk2 = wk @ kn_b + bk
    wv2 = wv * vn_g[None, :]
    bv2 = wv @ vn_b + bv

    # q/skip -> [D, Q] padded
    qT = np.zeros((D, QPAD), f32)
    qT[:, :Q] = q.reshape(D, Q)
    skipT = np.zeros((D, QPAD), f32)
    skipT[:, :Q] = skip.reshape(D, Q)

    # k/v -> rows [NKP, D]
    kRow = np.zeros((NKP, D), f32)
    kRow[:NK] = np.transpose(k, (0, 1, 3, 4, 2)).reshape(NK, D)
    vRow = np.zeros((NKP, D), f32)
    vRow[:NK] = np.transpose(v, (0, 1, 3, 4, 2)).reshape(NK, D)

    # W/vis -> transposed, padded; vis pad rows (queries) with 1 to avoid
    # a zero softmax denominator in the padding region
    Wp = np.zeros((QPAD, NKP), f32)
    Wp[:Q, :NK] = W[0]
    Cp = np.zeros((QPAD, NKP), f32)
    Cp[:Q, :NK] = vis[0]
    Cp[Q:, :] = 1.0

    # wproj head-major: wprojT [inner, D] -> [DH, HEADS, D]
    wprojT = np.ascontiguousarray(wproj.T)         # [inner, D]
    wprojTm = np.ascontiguousarray(
        wprojT.reshape(HEADS, DH, D).transpose(1, 0, 2))  # [DH, HEADS, D]

    shared = {
        "kR": kRow,
        "vR": vRow,
        "wqT": np.ascontiguousarray(wq2.T).astype(bf16),
        "wkT": np.ascontiguousarray(wk2.T).astype(bf16),
        "wvT": np.ascontiguousarray(wv2.T).astype(bf16),
        "bqm": np.ascontiguousarray(bq2.reshape(2, 64).T),
        "bkm": np.ascontiguousarray(bk2.reshape(2, 64).T),
        "wprojTm": wprojTm.astype(bf16),
        "bprojv": np.ascontiguousarray((wproj @ bv2 + bproj)[:, None]),
        "pre_gv": np.ascontiguousarray(pre_g[:, None]),
        "pre_bv": np.ascontiguousarray(pre_b[:, None]),
        "w1T": np.ascontiguousarray(w1.T).astype(bf16),
        "b1m": np.ascontiguousarray(b1.reshape(2, D).T),
        "w2Td": np.ascontiguousarray(w2.T.reshape(2, D, D)).astype(bf16),
        "b2v": np.ascontiguousarray(b2[:, None]),
        "post_gv": np.ascontiguousarray(post_g[:, None]),
        "post_bv": np.ascontiguousarray(post_b[:, None]),
    }

    in_maps = []
    for c in range(NCORES):
        sl = slice(c * QC, (c + 1) * QC)
        m = dict(shared)
        m["qT"] = np.ascontiguousarray(qT[:, sl])
        m["skipT"] = np.ascontiguousarray(skipT[:, sl])
        m["Wt"] = np.ascontiguousarray(Wp[sl].T).reshape(NKT, 128, QC).astype(bf16)
        m["Cm"] = np.ascontiguousarray(Cp[sl].T).reshape(NKT, 128, QC).astype(bf16)
        in_maps.append(m)
    return in_maps


def kernel(**inputs):
    from concourse.bass_utils import run_bass_kernel_spmd

    nc = _build()
    in_maps = _prep_inputs(inputs)
    res = run_bass_kernel_spmd(nc, in_maps, core_ids=list(range(NCORES)))
    outs = np.concatenate([r["outT"] for r in res.results], axis=1)  # [D, QPAD]
    return outs[:, :Q].reshape(1, D, HB, WB).astype(np.float32)
